# revision 1
# baseline (speedup 1.0000x reference)
"""CRF loss (forward-algorithm log-partition minus gold-path score) on 8 trn2 cores.

Strategy (data-parallel over B, 32 rows per core):
  Denominator: forward scan in probability space. With E = exp(transitions)
  as the PE stationary, each step is one matmul u = E^T @ alpha plus one DVE
  multiply alpha' = u * exp(emit_t - SHIFT). The constant SHIFT=log(128)+0.5
  cancels the expected per-step log-growth of the row-sum so fp32 stays in
  range; masking is handled by snapshotting log(row-sum) at every t >= 128
  and selecting t = len_b - 1 at the end via the mask's prefix structure
  (OH_len = maskf[t] - maskf[t+1]).
  Numerator: emission scores via one-hot matmuls accumulated over all
  (b, t-chunk) into a single PSUM tile, then a Frobenius product with I;
  transition scores from the same one-hot matmuls: PairCount = OH^T @ OH_next
  accumulated in PSUM, then a Frobenius product with the transitions table.
Output per core: scalar sum over its rows of (log_den - log_num); host
divides by B.
"""

import numpy as np
import ml_dtypes

B, T, C = 256, 512, 128
NCORES = 8
BL = B // NCORES
SHIFT = float(np.log(128.0) + 0.5)  # cancels E[log sum_j exp(em_j)] per step
TBL = C * C           # flat transitions table size
NPG = T * BL // 8     # gather pairs per 16-partition group (2048)
NHALF = NPG // 2      # per-gather indices (ISA limit ~1024 per indirect_copy)

_cache = {}


def _build_program():
    import concourse.bass as bass
    import concourse.bacc as bacc
    import concourse.tile as tile
    from concourse import mybir

    f32 = mybir.dt.float32
    bf16 = mybir.dt.bfloat16
    u16 = mybir.dt.uint16
    Alu = mybir.AluOpType
    Act = mybir.ActivationFunctionType
    Axis = mybir.AxisListType

    nc = bacc.Bacc(None)

    em_ctb = nc.dram_tensor("em_ctb", [C, T, BL], f32, kind="ExternalInput")
    em_btc = nc.dram_tensor("em_btc", [BL, T, C], f32, kind="ExternalInput")
    tagsm_tb = nc.dram_tensor("tagsm_tb", [T, BL], f32, kind="ExternalInput")
    tagsms_tb = nc.dram_tensor("tagsms_tb", [T, BL], f32, kind="ExternalInput")
    maskf_tb = nc.dram_tensor("maskf_tb", [T + 1, BL], f32, kind="ExternalInput")
    trans_in = nc.dram_tensor("trans", [C, C], f32, kind="ExternalInput")
    out_d = nc.dram_tensor("out", [1, 1], f32, kind="ExternalOutput")

    ident_in = nc.inline_tensor(np.eye(C, dtype=np.float32), name="ident")
    ones_in = nc.inline_tensor(np.ones((C, 1), np.float32), name="onescol")
    iota_in = nc.inline_tensor(
        np.tile(np.arange(C, dtype=np.float32), (C, 1)), name="iotarow"
    )

    NCH = T // 128          # 4 numerator t-chunks
    RS_K0 = 8               # rowsum chunks (16 t's each) start at t=128
    RS_K = 32               # ... through t=511

    with tile.TileContext(nc) as tc:
        with (
            tc.tile_pool(name="consts", bufs=1) as consts,
            tc.tile_pool(name="bigbuf", bufs=1) as bigbuf,
            tc.tile_pool(name="scanps", bufs=2, space="PSUM") as scanps,
            tc.tile_pool(name="accps", bufs=1, space="PSUM") as accps,
            tc.tile_pool(name="rsps", bufs=2, space="PSUM") as rsps,
            tc.tile_pool(name="oh", bufs=3) as ohpool,
            tc.tile_pool(name="emn", bufs=3) as emnpool,
            tc.tile_pool(name="logc", bufs=2) as logcpool,
            tc.tile_pool(name="dram", bufs=1, space="DRAM") as drampool,
        ):
            # ---------- constants / small inputs ----------
            trans_sb = consts.tile([C, C], f32)
            nc.sync.dma_start(out=trans_sb[:], in_=trans_in[:])
            E_sb = consts.tile([C, C], f32)
            nc.scalar.activation(out=E_sb[:], in_=trans_sb[:], func=Act.Exp)
            ident_sb = consts.tile([C, C], f32)
            nc.sync.dma_start(out=ident_sb[:], in_=ident_in[:])
            ones_sb = consts.tile([C, 1], f32)
            nc.sync.dma_start(out=ones_sb[:], in_=ones_in[:])
            iota_sb = consts.tile([C, C], f32)
            nc.sync.dma_start(out=iota_sb[:], in_=iota_in[:])
            neg_shift = consts.tile([C, 1], f32)
            nc.vector.memset(neg_shift[:], -SHIFT)

            tags_m = consts.tile([128, NCH, BL], f32)
            nc.sync.dma_start(
                out=tags_m[:],
                in_=tagsm_tb[:].rearrange("(h l) b -> l h b", l=128),
            )
            tags_ms = consts.tile([128, NCH, BL], f32)
            nc.sync.dma_start(
                out=tags_ms[:],
                in_=tagsms_tb[:].rearrange("(h l) b -> l h b", l=128),
            )
            maskf_t = consts.tile([128, NCH, BL], f32)
            nc.sync.dma_start(
                out=maskf_t[:],
                in_=maskf_tb[0:T, :].rearrange("(h l) b -> l h b", l=128),
            )
            maskf_s = consts.tile([128, NCH, BL], f32)
            nc.sync.dma_start(
                out=maskf_s[:],
                in_=maskf_tb[1 : T + 1, :].rearrange("(h l) b -> l h b", l=128),
            )

            # ---------- big buffers ----------
            exp_em = bigbuf.tile([C, T, BL], f32)
            nc.sync.dma_start(out=exp_em[:], in_=em_ctb[:])
            TCH = 64
            for k in range(T // TCH):
                nc.scalar.activation(
                    out=exp_em[:, k * TCH : (k + 1) * TCH, :],
                    in_=exp_em[:, k * TCH : (k + 1) * TCH, :],
                    func=Act.Exp, bias=neg_shift[:], scale=1.0,
                )
            S_all = bigbuf.tile([C, T, BL], f32)
            nc.vector.tensor_copy(out=S_all[:, 0, :], in_=exp_em[:, 0, :])

            # ---------- the scan ----------
            for t in range(1, T):
                u_ps = scanps.tile([C, BL], f32)
                nc.tensor.matmul(
                    u_ps[:], lhsT=E_sb[:], rhs=S_all[:, t - 1, :],
                    start=True, stop=True,
                )
                nc.vector.tensor_tensor(
                    out=S_all[:, t, :], in0=u_ps[:], in1=exp_em[:, t, :],
                    op=Alu.mult,
                )

            # ---------- row-sums + log snapshots (t >= 128) ----------
            scratch_log = drampool.tile([T * BL], f32)
            for k in range(RS_K0, RS_K):
                rs_ps = rsps.tile([1, 16 * BL], f32)
                nc.tensor.matmul(
                    rs_ps[:], lhsT=ones_sb[:, :1],
                    rhs=S_all[:, 16 * k : 16 * (k + 1), :],
                    start=True, stop=True,
                )
                logc = logcpool.tile([1, 16 * BL], f32)
                nc.scalar.activation(out=logc[:], in_=rs_ps[:], func=Act.Ln)
                nc.sync.dma_start(
                    out=scratch_log[16 * BL * k : 16 * BL * (k + 1)],
                    in_=logc[:],
                )

            # ---------- numerator: one-hot matmuls ----------
            emit_ps = accps.tile([C, C], f32)
            pair_ps = accps.tile([C, C], f32)
            for b in range(BL):
                for ch in range(NCH):
                    i = b * NCH + ch
                    em_nm = emnpool.tile([128, C], f32, tag="em_nm")
                    nc.sync.dma_start(
                        out=em_nm[:],
                        in_=em_btc[b, ch * 128 : (ch + 1) * 128, :],
                    )
                    em_bf = emnpool.tile([128, C], bf16, tag="em_bf")
                    nc.scalar.copy(out=em_bf[:], in_=em_nm[:])
                    oh = ohpool.tile([128, C], bf16, tag="oh")
                    nc.vector.tensor_tensor(
                        out=oh[:], in0=iota_sb[:],
                        in1=tags_m[:, ch, b : b + 1].to_broadcast([128, C]),
                        op=Alu.is_equal,
                    )
                    ohs = ohpool.tile([128, C], bf16, tag="ohs")
                    nc.vector.tensor_tensor(
                        out=ohs[:], in0=iota_sb[:],
                        in1=tags_ms[:, ch, b : b + 1].to_broadcast([128, C]),
                        op=Alu.is_equal,
                    )
                    nc.tensor.matmul(
                        emit_ps[:], lhsT=oh[:], rhs=em_bf[:],
                        start=(i == 0), stop=(i == BL * NCH - 1),
                        skip_group_check=True,
                    )
                    nc.tensor.matmul(
                        pair_ps[:], lhsT=oh[:], rhs=ohs[:],
                        start=(i == 0), stop=(i == BL * NCH - 1),
                        skip_group_check=True,
                    )

            # ---------- denominator combine ----------
            logRS = consts.tile([128, NCH - 1, BL], f32)
            nc.sync.dma_start(
                out=logRS[:],
                in_=scratch_log[128 * BL : T * BL].rearrange(
                    "(h l b) -> l h b", h=NCH - 1, l=128
                ),
            )
            ohl = consts.tile([128, NCH - 1, BL], f32)
            nc.vector.tensor_tensor(
                out=ohl[:], in0=maskf_t[:, 1:, :], in1=maskf_s[:, 1:, :],
                op=Alu.subtract,
            )
            den_acc = consts.tile([128, 1], f32)
            nc.vector.tensor_tensor(
                out=ohl[:], in0=ohl[:], in1=logRS[:], op=Alu.mult
            )
            nc.vector.tensor_reduce(
                out=den_acc[:], in_=ohl[:], axis=Axis.XY, op=Alu.add
            )
            L_acc = consts.tile([128, 1], f32)
            nc.vector.tensor_reduce(
                out=L_acc[:], in_=maskf_t[:], axis=Axis.XY, op=Alu.add
            )
            nc.scalar.mul(out=L_acc[:], in_=L_acc[:], mul=SHIFT)

            # ---------- numerator frobenius ----------
            emit_acc = consts.tile([128, 1], f32)
            nc.vector.tensor_tensor(
                out=emit_ps[:], in0=emit_ps[:], in1=ident_sb[:], op=Alu.mult
            )
            nc.vector.tensor_reduce(
                out=emit_acc[:], in_=emit_ps[:], axis=Axis.X, op=Alu.add
            )
            pair_acc = consts.tile([128, 1], f32)
            nc.vector.tensor_tensor(
                out=pair_ps[:], in0=pair_ps[:], in1=trans_sb[:], op=Alu.mult
            )
            nc.vector.tensor_reduce(
                out=pair_acc[:], in_=pair_ps[:], axis=Axis.X, op=Alu.add
            )

            # ---------- final reduce to scalar ----------
            fin = consts.tile([128, 1], f32)
            nc.vector.tensor_tensor(
                out=fin[:], in0=den_acc[:], in1=L_acc[:], op=Alu.add
            )
            nc.vector.tensor_tensor(
                out=fin[:], in0=fin[:], in1=emit_acc[:], op=Alu.subtract
            )
            nc.vector.tensor_tensor(
                out=fin[:], in0=fin[:], in1=pair_acc[:], op=Alu.subtract
            )
            fin_ps = rsps.tile([1, 1], f32, tag="fin")
            nc.tensor.matmul(
                fin_ps[:], lhsT=ones_sb[:, :1], rhs=fin[:],
                start=True, stop=True,
            )
            res_sb = consts.tile([1, 1], f32)
            nc.scalar.copy(out=res_sb[:], in_=fin_ps[:])
            nc.sync.dma_start(out=out_d[:], in_=res_sb[:])

    nc.compile()
    return nc


def _prep_inputs(emissions, tags, mask, transitions):
    em = np.ascontiguousarray(np.asarray(emissions), dtype=np.float32)
    tg = np.asarray(tags).astype(np.int32)
    mk = np.asarray(mask).astype(bool)
    tr = np.ascontiguousarray(np.asarray(transitions), dtype=np.float32)


    in_maps = []
    for core in range(NCORES):
        b0, b1 = core * BL, (core + 1) * BL
        em_c = em[b0:b1]
        tg_c = tg[b0:b1].T                            # [T, BL] int32
        mk_c = mk[b0:b1].T.astype(np.float32)         # [T, BL]
        pad_f = np.zeros((1, BL), np.float32)

        # masked tags (+1000 where mask off) for the one-hot builds
        tags_m = (tg_c + 1000.0 * (1.0 - mk_c)).astype(np.float32)
        tg_next = np.vstack([tg_c[1:], np.zeros((1, BL), np.int32)])
        mk_next = np.vstack([mk_c[1:], pad_f])
        tags_ms = (tg_next + 1000.0 * (1.0 - mk_next)).astype(np.float32)

        in_maps.append({
            "em_ctb": np.ascontiguousarray(em_c.transpose(2, 1, 0)),
            "em_btc": np.ascontiguousarray(em_c),
            "tagsm_tb": np.ascontiguousarray(tags_m),
            "tagsms_tb": np.ascontiguousarray(tags_ms),
            "maskf_tb": np.ascontiguousarray(np.vstack([mk_c, pad_f])),
            "trans": tr,
        })
    return in_maps


def kernel(emissions, tags, mask, transitions, _want_results=False, **_run_kw):
    from concourse.bass_utils import run_bass_kernel_spmd

    if "nc" not in _cache:
        _cache["nc"] = _build_program()
    nc = _cache["nc"]

    in_maps = _prep_inputs(emissions, tags, mask, transitions)
    res = run_bass_kernel_spmd(nc, in_maps, core_ids=list(range(NCORES)), **_run_kw)
    total = sum(float(r["out"][0, 0]) for r in res.results)
    out = np.float32(total / B)
    if _want_results:
        return out, res
    return out



# revision 4
# speedup vs baseline: 1.8544x; 1.8544x over previous
"""CRF loss (forward-algorithm log-partition minus gold-path score) on 8 trn2 cores.

Strategy (data-parallel over B, 32 rows per core):
  Denominator: forward scan in probability space, all-bf16 on the PE/DVE
  path. With E = exp(transitions) as the bf16 PE stationary, each step is
  one matmul u = E^T @ alpha plus one DVE multiply alpha' = u * exp(emit_t
  - SHIFT) (PSUM fp32 -> bf16 SBUF). The constant SHIFT=log(128)+0.5
  cancels the expected per-step log-growth of the row-sum so the values
  stay in range; masking is handled by snapshotting log(row-sum) at every
  t >= 128 and selecting t = len_b - 1 at the end via the mask's prefix
  structure (OH_len = maskf[t] - maskf[t+1]).
  The 32 batch rows are split into two independent 16-row chains (A/B)
  so the serial matmul->multiply->matmul latency of one chain hides
  under the other's work.
  Numerator: one-hot matmuls, one per (b, t-chunk): lhsT = OH(tags),
  rhs = [emissions_chunk | OH(tags_next)] concatenated [128, 256],
  accumulated over all 128 iterations into a single PSUM tile; a single
  Frobenius product with [I | transitions] then yields emit + trans
  scores summed. One-hot builds are batched is_equal compares (oh on
  DVE, ohs on GpSimd to keep DVE free for the scan).
Output per core: scalar sum over its rows of (log_den - log_num); host
divides by B.
"""

import numpy as np
import ml_dtypes

B, T, C = 256, 512, 128
NCORES = 8
BL = B // NCORES
HB = BL // 2          # rows per scan chain
SHIFT = float(np.log(128.0) + 0.5)  # cancels E[log sum_j exp(em_j)] per step
NCH = T // 128        # 4 numerator t-chunks
RS_K0 = 8             # rowsum chunks (16 t's each) start at t=128
RS_K = 32             # ... through t=511

_cache = {}


def _build_program():
    import concourse.bass as bass
    import concourse.bacc as bacc
    import concourse.tile as tile
    from concourse import mybir

    f32 = mybir.dt.float32
    bf16 = mybir.dt.bfloat16
    Alu = mybir.AluOpType
    Act = mybir.ActivationFunctionType
    Axis = mybir.AxisListType

    nc = bacc.Bacc(None)

    em_ctb = nc.dram_tensor("em_ctb", [C, T, BL], bf16, kind="ExternalInput")
    em_btc = nc.dram_tensor("em_btc", [BL, T, C], bf16, kind="ExternalInput")
    tagsm_tb = nc.dram_tensor("tagsm_tb", [T, BL], f32, kind="ExternalInput")
    tagsms_tb = nc.dram_tensor("tagsms_tb", [T, BL], f32, kind="ExternalInput")
    maskf_tb = nc.dram_tensor("maskf_tb", [T + 1, BL], f32, kind="ExternalInput")
    trans_in = nc.dram_tensor("trans", [C, C], f32, kind="ExternalInput")
    out_d = nc.dram_tensor("out", [1, 1], f32, kind="ExternalOutput")

    ident_in = nc.inline_tensor(np.eye(C, dtype=np.float32), name="ident")
    onesb_in = nc.inline_tensor(
        np.ones((C, 1), ml_dtypes.bfloat16), name="onesbf"
    )
    onesf_in = nc.inline_tensor(np.ones((C, 1), np.float32), name="onesf")
    iota4_in = nc.inline_tensor(
        np.broadcast_to(
            np.arange(C, dtype=np.float32), (C, NCH, C)
        ).copy(),
        name="iota4",
    )

    NRS = RS_K - RS_K0            # rowsum chunks per chain (24)

    with tile.TileContext(nc) as tc:
        with (
            tc.tile_pool(name="consts", bufs=1) as consts,
            tc.tile_pool(name="bigbuf", bufs=1) as bigbuf,
            tc.tile_pool(name="scanpsA", bufs=2, space="PSUM") as scanpsA,
            tc.tile_pool(name="scanpsB", bufs=2, space="PSUM") as scanpsB,
            tc.tile_pool(name="accps", bufs=1, space="PSUM") as accps,
            tc.tile_pool(name="rsps", bufs=2, space="PSUM") as rsps,
            tc.tile_pool(name="oh", bufs=3) as ohpool,
            tc.tile_pool(name="combo", bufs=3) as combopool,
            tc.tile_pool(name="logc", bufs=2) as logcpool,
            tc.tile_pool(name="dram", bufs=1, space="DRAM") as drampool,
        ):
            # ---------- constants / small inputs ----------
            # idtr = [I | transitions]  (fp32) for the final Frobenius
            idtr = consts.tile([C, 2 * C], f32)
            nc.sync.dma_start(out=idtr[:, 0:C], in_=ident_in[:])
            nc.sync.dma_start(out=idtr[:, C : 2 * C], in_=trans_in[:])
            E_bf = consts.tile([C, C], bf16)
            nc.scalar.activation(out=E_bf[:], in_=idtr[:, C : 2 * C], func=Act.Exp)
            ones_bf = consts.tile([C, 1], bf16)
            nc.sync.dma_start(out=ones_bf[:], in_=onesb_in[:])
            ones_f = consts.tile([C, 1], f32)
            nc.sync.dma_start(out=ones_f[:], in_=onesf_in[:])
            iota4_sb = consts.tile([C, NCH, C], f32)
            nc.sync.dma_start(out=iota4_sb[:], in_=iota4_in[:])
            neg_shift = consts.tile([C, 1], f32)
            nc.vector.memset(neg_shift[:], -SHIFT)

            tags_m = consts.tile([128, NCH, BL], f32)
            nc.sync.dma_start(
                out=tags_m[:],
                in_=tagsm_tb[:].rearrange("(h l) b -> l h b", l=128),
            )
            tags_ms = consts.tile([128, NCH, BL], f32)
            nc.sync.dma_start(
                out=tags_ms[:],
                in_=tagsms_tb[:].rearrange("(h l) b -> l h b", l=128),
            )
            maskf_t = consts.tile([128, NCH, BL], f32)
            nc.sync.dma_start(
                out=maskf_t[:],
                in_=maskf_tb[0:T, :].rearrange("(h l) b -> l h b", l=128),
            )
            maskf_s = consts.tile([128, NCH, BL], f32)
            nc.sync.dma_start(
                out=maskf_s[:],
                in_=maskf_tb[1 : T + 1, :].rearrange("(h l) b -> l h b", l=128),
            )

            # ---------- big buffers ----------
            exp_em = bigbuf.tile([C, T, BL], bf16)
            nc.sync.dma_start(out=exp_em[:], in_=em_ctb[:])
            TCH = 64
            for k in range(T // TCH):
                nc.scalar.activation(
                    out=exp_em[:, k * TCH : (k + 1) * TCH, :],
                    in_=exp_em[:, k * TCH : (k + 1) * TCH, :],
                    func=Act.Exp, bias=neg_shift[:], scale=1.0,
                )
            S_A = bigbuf.tile([C, T, HB], bf16)
            S_B = bigbuf.tile([C, T, HB], bf16)

            # ---------- the scan (two independent chains) ----------
            for t in range(1, T):
                rhs_A = exp_em[:, 0, 0:HB] if t == 1 else S_A[:, t - 1, :]
                rhs_B = exp_em[:, 0, HB:BL] if t == 1 else S_B[:, t - 1, :]
                uA = scanpsA.tile([C, HB], f32, tag="uA")
                nc.tensor.matmul(
                    uA[:], lhsT=E_bf[:], rhs=rhs_A, start=True, stop=True
                )
                uB = scanpsB.tile([C, HB], f32, tag="uB")
                nc.tensor.matmul(
                    uB[:], lhsT=E_bf[:], rhs=rhs_B, start=True, stop=True
                )
                nc.vector.tensor_tensor(
                    out=S_A[:, t, :], in0=uA[:], in1=exp_em[:, t, 0:HB],
                    op=Alu.mult,
                )
                nc.vector.tensor_tensor(
                    out=S_B[:, t, :], in0=uB[:], in1=exp_em[:, t, HB:BL],
                    op=Alu.mult,
                )

            # ---------- row-sums + log snapshots (t >= 128) ----------
            # scratch layout per chain: [h(3), l(128), b(HB)] fp32
            scratch_log = drampool.tile([2, 3, 128, HB], f32)
            for k in range(RS_K0, RS_K):
                h_k = (16 * k - 128) // 128
                lp = (16 * k - 128) % 128
                for ci, S_X in ((0, S_A), (1, S_B)):
                    rs_ps = rsps.tile([1, 16 * HB], f32, tag="rs")
                    nc.tensor.matmul(
                        rs_ps[:], lhsT=ones_bf[:, :1],
                        rhs=S_X[:, 16 * k : 16 * (k + 1), :],
                        start=True, stop=True,
                    )
                    logc = logcpool.tile([1, 16 * HB], f32)
                    nc.scalar.activation(out=logc[:], in_=rs_ps[:], func=Act.Ln)
                    nc.sync.dma_start(
                        out=scratch_log[ci, h_k, lp : lp + 16, :],
                        in_=logc[:],
                    )

            # ---------- numerator: one-hot matmuls ----------
            acc_ps = accps.tile([C, 2 * C], f32)
            for b in range(BL):
                oh = ohpool.tile([128, NCH, C], bf16, tag="oh")
                nc.vector.tensor_tensor(
                    out=oh[:], in0=iota4_sb[:],
                    in1=tags_m[:, :, b : b + 1].to_broadcast([128, NCH, C]),
                    op=Alu.is_equal,
                )
                combo = combopool.tile([128, NCH, 2 * C], bf16, tag="combo")
                nc.sync.dma_start(
                    out=combo[:, :, 0:C],
                    in_=em_btc[b].rearrange("(h l) c -> l h c", l=128),
                )
                nc.vector.tensor_tensor(
                    out=combo[:, :, C : 2 * C], in0=iota4_sb[:],
                    in1=tags_ms[:, :, b : b + 1].to_broadcast([128, NCH, C]),
                    op=Alu.is_equal,
                )
                for ch in range(NCH):
                    i = b * NCH + ch
                    nc.tensor.matmul(
                        acc_ps[:], lhsT=oh[:, ch, :], rhs=combo[:, ch, :],
                        start=(i == 0), stop=(i == BL * NCH - 1),
                        skip_group_check=True,
                    )

            # ---------- denominator combine ----------
            den_accs = []
            for ci in range(2):
                logRS = consts.tile([128, 3, HB], f32, tag=f"logRS{ci}")
                nc.sync.dma_start(
                    out=logRS[:],
                    in_=scratch_log[ci].rearrange("h l b -> l h b"),
                )
                bs = ci * HB
                ohl = consts.tile([128, 3, HB], f32, tag=f"ohl{ci}")
                nc.vector.tensor_tensor(
                    out=ohl[:], in0=maskf_t[:, 1:, bs : bs + HB],
                    in1=maskf_s[:, 1:, bs : bs + HB],
                    op=Alu.subtract,
                )
                nc.vector.tensor_tensor(
                    out=ohl[:], in0=ohl[:], in1=logRS[:], op=Alu.mult
                )
                dacc = consts.tile([128, 1], f32, tag=f"dacc{ci}")
                nc.vector.tensor_reduce(
                    out=dacc[:], in_=ohl[:], axis=Axis.XY, op=Alu.add
                )
                den_accs.append(dacc)
            L_acc = consts.tile([128, 1], f32)
            nc.vector.tensor_reduce(
                out=L_acc[:], in_=maskf_t[:], axis=Axis.XY, op=Alu.add
            )
            nc.scalar.mul(out=L_acc[:], in_=L_acc[:], mul=SHIFT)

            # ---------- numerator frobenius ([I | trans] in one shot) ----------
            frob = consts.tile([C, 2 * C], f32)
            nc.vector.tensor_tensor(
                out=frob[:], in0=acc_ps[:], in1=idtr[:], op=Alu.mult
            )
            num_acc = consts.tile([128, 1], f32)
            nc.vector.tensor_reduce(
                out=num_acc[:], in_=frob[:], axis=Axis.X, op=Alu.add
            )

            # ---------- final reduce to scalar ----------
            fin = consts.tile([128, 1], f32)
            nc.vector.tensor_tensor(
                out=fin[:], in0=den_accs[0][:], in1=den_accs[1][:], op=Alu.add
            )
            nc.vector.tensor_tensor(
                out=fin[:], in0=fin[:], in1=L_acc[:], op=Alu.add
            )
            nc.vector.tensor_tensor(
                out=fin[:], in0=fin[:], in1=num_acc[:], op=Alu.subtract
            )
            fin_ps = rsps.tile([1, 1], f32, tag="rs")
            nc.tensor.matmul(
                fin_ps[:], lhsT=ones_f[:, :1], rhs=fin[:],
                start=True, stop=True,
            )
            res_sb = consts.tile([1, 1], f32)
            nc.scalar.copy(out=res_sb[:], in_=fin_ps[:])
            nc.sync.dma_start(out=out_d[:], in_=res_sb[:])

    nc.compile()
    return nc


def _prep_inputs(emissions, tags, mask, transitions):
    em = np.asarray(emissions)
    tg = np.asarray(tags).astype(np.int32)
    mk = np.asarray(mask).astype(bool)
    tr = np.ascontiguousarray(np.asarray(transitions), dtype=np.float32)

    in_maps = []
    for core in range(NCORES):
        b0, b1 = core * BL, (core + 1) * BL
        em_c = np.asarray(em[b0:b1], dtype=np.float32)
        tg_c = tg[b0:b1].T                            # [T, BL] int32
        mk_c = mk[b0:b1].T.astype(np.float32)         # [T, BL]
        pad_f = np.zeros((1, BL), np.float32)

        # masked tags (+1000 where mask off) for the one-hot builds
        tags_m = (tg_c + 1000.0 * (1.0 - mk_c)).astype(np.float32)
        tg_next = np.vstack([tg_c[1:], np.zeros((1, BL), np.int32)])
        mk_next = np.vstack([mk_c[1:], pad_f])
        tags_ms = (tg_next + 1000.0 * (1.0 - mk_next)).astype(np.float32)

        in_maps.append({
            "em_ctb": np.ascontiguousarray(
                em_c.transpose(2, 1, 0)
            ).astype(ml_dtypes.bfloat16),
            "em_btc": np.ascontiguousarray(em_c).astype(ml_dtypes.bfloat16),
            "tagsm_tb": np.ascontiguousarray(tags_m),
            "tagsms_tb": np.ascontiguousarray(tags_ms),
            "maskf_tb": np.ascontiguousarray(np.vstack([mk_c, pad_f])),
            "trans": tr,
        })
    return in_maps


def kernel(emissions, tags, mask, transitions, _want_results=False, **_run_kw):
    from concourse.bass_utils import run_bass_kernel_spmd

    if "nc" not in _cache:
        _cache["nc"] = _build_program()
    nc = _cache["nc"]

    in_maps = _prep_inputs(emissions, tags, mask, transitions)
    res = run_bass_kernel_spmd(nc, in_maps, core_ids=list(range(NCORES)), **_run_kw)
    total = sum(float(r["out"][0, 0]) for r in res.results)
    out = np.float32(total / B)
    if _want_results:
        return out, res
    return out


# revision 5
# speedup vs baseline: 2.0503x; 1.1056x over previous
"""CRF loss (forward-algorithm log-partition minus gold-path score) on 8 trn2 cores.

Strategy (data-parallel over B, 32 rows per core):

  Denominator via a split scan that halves the serial-latency chain:
    Z_b = 1^T M_l .. M_1 a_0  (l = len_b - 1, M_t = diag(e_t) E^T,
    e_t = exp(emit_t - SHIFT), E = exp(transitions), a_0 = e_0).
    * forward half  (t = 1..255):   a <- e_t * (E^T a)         [true alpha_255]
    * backward half (t = 511..256): r <- E (e~_t * r) + (1-m_t)
      where e~_t is e_t with masked steps zeroed (host bakes -1000 into
      the masked emissions so exp underflows to 0) and m_t is the mask.
      Masked steps therefore compute r <- 0 + 1, the correct "inactive"
      suffix state, with no select op. Both halves run concurrently
      (independent chains); Z_b = <r_256, a_255> per row, one dot.
    The SHIFT normalization contributes exactly SHIFT*len_b per row,
    added back on the host (lengths are known there).
  All scan arithmetic is bf16 on the PE/DVE path with fp32 PSUM.

  Numerator: one-hot matmuls, one per (b, t-chunk): lhsT = OH(tags),
  rhs = [emissions_chunk | OH(tags_next)] concatenated [128, 256],
  accumulated over all 128 iterations into a single PSUM tile; a single
  Frobenius product with [I | transitions] then yields emit + trans
  scores summed.

Output per core: scalar sum over its rows of (log Z~_b - log_num_b);
host adds SHIFT*sum(len)/B and divides by B.
"""

import numpy as np
import ml_dtypes

B, T, C = 256, 512, 128
NCORES = 8
BL = B // NCORES
TH = T // 2           # split point: fwd covers t<TH via alpha, bwd t>=TH
SHIFT = float(np.log(128.0) + 0.5)  # cancels E[log sum_j exp(em_j)] per step
NCH = T // 128        # 4 numerator t-chunks

_cache = {}


def _build_program():
    import concourse.bass as bass
    import concourse.bacc as bacc
    import concourse.tile as tile
    from concourse import mybir

    f32 = mybir.dt.float32
    bf16 = mybir.dt.bfloat16
    Alu = mybir.AluOpType
    Act = mybir.ActivationFunctionType
    Axis = mybir.AxisListType

    nc = bacc.Bacc(None)

    em_ctb = nc.dram_tensor("em_ctb", [C, T, BL], bf16, kind="ExternalInput")
    em_btc = nc.dram_tensor("em_btc", [BL, T, C], bf16, kind="ExternalInput")
    tagsm_tb = nc.dram_tensor("tagsm_tb", [T, BL], f32, kind="ExternalInput")
    tagsms_tb = nc.dram_tensor("tagsms_tb", [T, BL], f32, kind="ExternalInput")
    onem_tb = nc.dram_tensor("onem_tb", [T - TH, BL], bf16, kind="ExternalInput")
    trans_in = nc.dram_tensor("trans", [C, C], f32, kind="ExternalInput")
    transT_in = nc.dram_tensor("transT", [C, C], f32, kind="ExternalInput")
    out_d = nc.dram_tensor("out", [1, 1], f32, kind="ExternalOutput")

    ident_in = nc.inline_tensor(np.eye(C, dtype=np.float32), name="ident")
    onesb_in = nc.inline_tensor(
        np.ones((C, 1), ml_dtypes.bfloat16), name="onesbf"
    )
    onesf_in = nc.inline_tensor(np.ones((C, 1), np.float32), name="onesf")
    iota4_in = nc.inline_tensor(
        np.broadcast_to(
            np.arange(C, dtype=np.float32), (C, NCH, C)
        ).copy(),
        name="iota4",
    )

    with tile.TileContext(nc) as tc:
        with (
            tc.tile_pool(name="consts", bufs=1) as consts,
            tc.tile_pool(name="bigbuf", bufs=1) as bigbuf,
            tc.tile_pool(name="fa", bufs=3) as fapool,
            tc.tile_pool(name="rr", bufs=3) as rrpool,
            tc.tile_pool(name="ww", bufs=3) as wwpool,
            tc.tile_pool(name="fps", bufs=2, space="PSUM") as fps,
            tc.tile_pool(name="bps", bufs=2, space="PSUM") as bps,
            tc.tile_pool(name="accps", bufs=1, space="PSUM") as accps,
            tc.tile_pool(name="rsps", bufs=2, space="PSUM") as rsps,
            tc.tile_pool(name="oh", bufs=3) as ohpool,
            tc.tile_pool(name="combo", bufs=3) as combopool,
        ):
            # ---------- constants / small inputs ----------
            # idtr = [I | transitions]  (fp32) for the final Frobenius
            idtr = consts.tile([C, 2 * C], f32)
            nc.sync.dma_start(out=idtr[:, 0:C], in_=ident_in[:])
            nc.sync.dma_start(out=idtr[:, C : 2 * C], in_=trans_in[:])
            E_bf = consts.tile([C, C], bf16)
            nc.scalar.activation(out=E_bf[:], in_=idtr[:, C : 2 * C], func=Act.Exp)
            transT_sb = consts.tile([C, C], f32)
            nc.sync.dma_start(out=transT_sb[:], in_=transT_in[:])
            ET_bf = consts.tile([C, C], bf16)
            nc.scalar.activation(out=ET_bf[:], in_=transT_sb[:], func=Act.Exp)
            ones_bf = consts.tile([C, 1], bf16)
            nc.sync.dma_start(out=ones_bf[:], in_=onesb_in[:])
            ones_f = consts.tile([C, 1], f32)
            nc.sync.dma_start(out=ones_f[:], in_=onesf_in[:])
            iota4_sb = consts.tile([C, NCH, C], f32)
            nc.sync.dma_start(out=iota4_sb[:], in_=iota4_in[:])
            neg_shift = consts.tile([C, 1], f32)
            nc.vector.memset(neg_shift[:], -SHIFT)

            tags_m = consts.tile([128, NCH, BL], f32)
            nc.sync.dma_start(
                out=tags_m[:],
                in_=tagsm_tb[:].rearrange("(h l) b -> l h b", l=128),
            )
            tags_ms = consts.tile([128, NCH, BL], f32)
            nc.sync.dma_start(
                out=tags_ms[:],
                in_=tagsms_tb[:].rearrange("(h l) b -> l h b", l=128),
            )
            # 1 - mask for t in [TH, T), replicated across all 128 partitions
            onem_sb = consts.tile([128, T - TH, BL], bf16)
            nc.sync.dma_start(
                out=onem_sb[:],
                in_=onem_tb[:].partition_broadcast(128),
            )

            # ---------- emissions: exp(em - SHIFT), bf16 ----------
            exp_em = bigbuf.tile([C, T, BL], bf16)
            nc.sync.dma_start(out=exp_em[:], in_=em_ctb[:])
            TCH = 64
            for k in range(T // TCH):
                nc.scalar.activation(
                    out=exp_em[:, k * TCH : (k + 1) * TCH, :],
                    in_=exp_em[:, k * TCH : (k + 1) * TCH, :],
                    func=Act.Exp, bias=neg_shift[:], scale=1.0,
                )

            # backward initial suffix state: all-ones
            r_prev = consts.tile([C, BL], bf16)
            nc.vector.memset(r_prev[:], 1.0)

            # ---------- the split scan ----------
            # fwd: a(t) = e_t * (E^T a(t-1)),  t = 1..TH-1,  a(0) = e_0
            # bwd: r(t) = E (e~_t * r(t+1)) + (1 - m_t),  t = T-1..TH
            fa_prev = None
            for k in range(T - TH):
                tf = 1 + k            # 1 .. 255 (skipped when k == T-TH-1)
                tb = T - 1 - k        # 511 .. 256
                # backward: w = e~_tb * r ; u = ET^T w ; r' = u + (1-m_tb)
                w = wwpool.tile([C, BL], bf16, tag="w")
                nc.vector.tensor_tensor(
                    out=w[:], in0=r_prev[:], in1=exp_em[:, tb, :], op=Alu.mult
                )
                ub = bps.tile([C, BL], f32, tag="ub")
                nc.tensor.matmul(
                    ub[:], lhsT=ET_bf[:], rhs=w[:], start=True, stop=True
                )
                r_new = rrpool.tile([C, BL], bf16, tag="r")
                nc.vector.tensor_tensor(
                    out=r_new[:], in0=ub[:], in1=onem_sb[:, tb - TH, :],
                    op=Alu.add,
                )
                r_prev = r_new
                # forward
                if tf < TH:
                    rhs_f = exp_em[:, 0, :] if tf == 1 else fa_prev[:]
                    uf = fps.tile([C, BL], f32, tag="uf")
                    nc.tensor.matmul(
                        uf[:], lhsT=E_bf[:], rhs=rhs_f, start=True, stop=True
                    )
                    fa_new = fapool.tile([C, BL], bf16, tag="fa")
                    nc.vector.tensor_tensor(
                        out=fa_new[:], in0=uf[:], in1=exp_em[:, tf, :],
                        op=Alu.mult,
                    )
                    fa_prev = fa_new

            # ---------- denominator: per-row dot + log ----------
            d = consts.tile([C, BL], bf16)
            nc.vector.tensor_tensor(
                out=d[:], in0=fa_prev[:], in1=r_prev[:], op=Alu.mult
            )
            dot_ps = rsps.tile([1, BL], f32, tag="rs")
            nc.tensor.matmul(
                dot_ps[:], lhsT=ones_bf[:, :1], rhs=d[:], start=True, stop=True
            )
            logd = consts.tile([1, BL], f32)
            nc.scalar.activation(out=logd[:], in_=dot_ps[:], func=Act.Ln)
            den_s = consts.tile([1, 1], f32)
            nc.vector.tensor_reduce(
                out=den_s[:], in_=logd[:], axis=Axis.X, op=Alu.add
            )

            # ---------- numerator: one-hot matmuls ----------
            acc_ps = accps.tile([C, 2 * C], f32)
            for b in range(BL):
                oh = ohpool.tile([128, NCH, C], bf16, tag="oh")
                nc.vector.tensor_tensor(
                    out=oh[:], in0=iota4_sb[:],
                    in1=tags_m[:, :, b : b + 1].to_broadcast([128, NCH, C]),
                    op=Alu.is_equal,
                )
                combo = combopool.tile([128, NCH, 2 * C], bf16, tag="combo")
                nc.sync.dma_start(
                    out=combo[:, :, 0:C],
                    in_=em_btc[b].rearrange("(h l) c -> l h c", l=128),
                )
                nc.vector.tensor_tensor(
                    out=combo[:, :, C : 2 * C], in0=iota4_sb[:],
                    in1=tags_ms[:, :, b : b + 1].to_broadcast([128, NCH, C]),
                    op=Alu.is_equal,
                )
                for ch in range(NCH):
                    i = b * NCH + ch
                    nc.tensor.matmul(
                        acc_ps[:], lhsT=oh[:, ch, :], rhs=combo[:, ch, :],
                        start=(i == 0), stop=(i == BL * NCH - 1),
                        skip_group_check=True,
                    )

            # ---------- numerator frobenius ([I | trans] in one shot) ----------
            frob = consts.tile([C, 2 * C], f32)
            nc.vector.tensor_tensor(
                out=frob[:], in0=acc_ps[:], in1=idtr[:], op=Alu.mult
            )
            num_acc = consts.tile([128, 1], f32)
            nc.vector.tensor_reduce(
                out=num_acc[:], in_=frob[:], axis=Axis.X, op=Alu.add
            )
            num_ps = rsps.tile([1, 1], f32, tag="rs")
            nc.tensor.matmul(
                num_ps[:], lhsT=ones_f[:, :1], rhs=num_acc[:],
                start=True, stop=True,
            )

            # ---------- final scalar ----------
            res_sb = consts.tile([1, 1], f32)
            nc.vector.tensor_tensor(
                out=res_sb[:], in0=den_s[:], in1=num_ps[:], op=Alu.subtract
            )
            nc.sync.dma_start(out=out_d[:], in_=res_sb[:])

    nc.compile()
    return nc


def _prep_inputs(emissions, tags, mask, transitions):
    em = np.asarray(emissions)
    tg = np.asarray(tags).astype(np.int32)
    mk = np.asarray(mask).astype(bool)
    tr = np.ascontiguousarray(np.asarray(transitions), dtype=np.float32)
    trT = np.ascontiguousarray(tr.T)

    in_maps = []
    for core in range(NCORES):
        b0, b1 = core * BL, (core + 1) * BL
        em_c = np.asarray(em[b0:b1], dtype=np.float32)
        mk_c3 = mk[b0:b1][:, :, None]                 # [BL, T, 1]
        em_masked = np.where(mk_c3, em_c, -1000.0).astype(np.float32)
        tg_c = tg[b0:b1].T                            # [T, BL] int32
        mk_c = mk[b0:b1].T.astype(np.float32)         # [T, BL]
        pad_f = np.zeros((1, BL), np.float32)

        # masked tags (+1000 where mask off) for the one-hot builds
        tags_m = (tg_c + 1000.0 * (1.0 - mk_c)).astype(np.float32)
        tg_next = np.vstack([tg_c[1:], np.zeros((1, BL), np.int32)])
        mk_next = np.vstack([mk_c[1:], pad_f])
        tags_ms = (tg_next + 1000.0 * (1.0 - mk_next)).astype(np.float32)

        in_maps.append({
            "em_ctb": np.ascontiguousarray(
                em_masked.transpose(2, 1, 0)
            ).astype(ml_dtypes.bfloat16),
            "em_btc": np.ascontiguousarray(em_c).astype(ml_dtypes.bfloat16),
            "tagsm_tb": np.ascontiguousarray(tags_m),
            "tagsms_tb": np.ascontiguousarray(tags_ms),
            "onem_tb": np.ascontiguousarray(
                1.0 - mk_c[TH:T]
            ).astype(ml_dtypes.bfloat16),
            "trans": tr,
            "transT": trT,
        })
    return in_maps


def kernel(emissions, tags, mask, transitions, _want_results=False, **_run_kw):
    from concourse.bass_utils import run_bass_kernel_spmd

    if "nc" not in _cache:
        _cache["nc"] = _build_program()
    nc = _cache["nc"]

    in_maps = _prep_inputs(emissions, tags, mask, transitions)
    res = run_bass_kernel_spmd(nc, in_maps, core_ids=list(range(NCORES)), **_run_kw)
    total = sum(float(r["out"][0, 0]) for r in res.results)
    lengths_total = int(np.asarray(mask).astype(np.int64).sum())
    out = np.float32((total + SHIFT * lengths_total) / B)
    if _want_results:
        return out, res
    return out


# revision 10
# speedup vs baseline: 2.5022x; 1.2204x over previous
"""CRF loss (forward-algorithm log-partition minus gold-path score) on 8 trn2 cores.

Strategy (data-parallel over B, 32 rows per core):

  Denominator via a split scan that halves the serial-latency chain:
    Z_b = 1^T M_l .. M_1 a_0  (l = len_b - 1, M_t = diag(e_t) E^T,
    e_t = exp(emit_t - SHIFT), E = exp(transitions), a_0 = e_0).
    * forward half  (t = 1..255):   a <- e_t * (E^T a)         [true alpha_255]
    * backward half (t = 511..256): r <- E (e~_t * r) + (1-m_t)
      where e~_t is e_t with masked steps zeroed (host bakes -1000 into
      the masked emissions so exp underflows to 0) and m_t is the mask.
      Masked steps therefore compute r <- 0 + 1, the correct "inactive"
      suffix state, with no select op. Both halves run concurrently
      (independent chains); Z_b = <r_256, a_255> per row, one dot.
    The SHIFT normalization contributes exactly SHIFT*len_b per row,
    added back on the host (lengths are known there).
  All scan arithmetic is bf16 on the PE/DVE path with fp32 PSUM.

  Numerator: one-hot matmuls, one per (b, t-chunk): lhsT = OH(tags),
  rhs = [emissions_chunk | OH(tags_next)] concatenated [128, 256],
  accumulated over all 128 iterations into a single PSUM tile; a single
  Frobenius product with [I | transitions] then yields emit + trans
  scores summed.

Output per core: scalar sum over its rows of (log Z~_b - log_num_b);
host adds SHIFT*sum(len)/B and divides by B.
"""

import numpy as np
import ml_dtypes

B, T, C = 256, 512, 128
NCORES = 8
BL = B // NCORES
TH = T // 2           # split point: fwd covers t<TH via alpha, bwd t>=TH
SHIFT = float(np.log(128.0) + 0.5)  # cancels E[log sum_j exp(em_j)] per step
NCH = T // 128        # 4 numerator t-chunks

_cache = {}


def _build_program():
    import concourse.bass as bass
    import concourse.bacc as bacc
    import concourse.tile as tile
    from concourse import mybir

    f32 = mybir.dt.float32
    bf16 = mybir.dt.bfloat16
    Alu = mybir.AluOpType
    Act = mybir.ActivationFunctionType
    Axis = mybir.AxisListType

    nc = bacc.Bacc(None)

    em_ctb = nc.dram_tensor("em_ctb", [C, T, BL], bf16, kind="ExternalInput")
    em_btc = nc.dram_tensor("em_btc", [BL, T, C], bf16, kind="ExternalInput")
    tagsm_tb = nc.dram_tensor("tagsm_tb", [T, BL], f32, kind="ExternalInput")
    tagsms_tb = nc.dram_tensor("tagsms_tb", [T, BL], f32, kind="ExternalInput")
    onem_tb = nc.dram_tensor("onem_tb", [T - TH, BL], bf16, kind="ExternalInput")
    trans_in = nc.dram_tensor("trans", [C, C], f32, kind="ExternalInput")
    transT_in = nc.dram_tensor("transT", [C, C], f32, kind="ExternalInput")
    out_d = nc.dram_tensor("out", [1, 1], f32, kind="ExternalOutput")

    ident_in = nc.inline_tensor(np.eye(C, dtype=np.float32), name="ident")
    onesb_in = nc.inline_tensor(
        np.ones((C, 1), ml_dtypes.bfloat16), name="onesbf"
    )
    onesrow_in = nc.inline_tensor(
        np.ones((1, C), ml_dtypes.bfloat16), name="onesrow"
    )
    onesf_in = nc.inline_tensor(np.ones((C, 1), np.float32), name="onesf")
    iota4_in = nc.inline_tensor(
        np.broadcast_to(
            np.arange(C, dtype=np.float32), (C, NCH, C)
        ).copy(),
        name="iota4",
    )

    with tile.TileContext(nc) as tc:
        with (
            tc.tile_pool(name="consts", bufs=1) as consts,
            tc.tile_pool(name="bigbuf", bufs=1) as bigbuf,
            tc.tile_pool(name="fa", bufs=3) as fapool,
            tc.tile_pool(name="rr", bufs=3) as rrpool,
            tc.tile_pool(name="ww", bufs=3) as wwpool,
            tc.tile_pool(name="fps", bufs=2, space="PSUM") as fps,
            tc.tile_pool(name="bps", bufs=2, space="PSUM") as bps,
            tc.tile_pool(name="accps", bufs=1, space="PSUM") as accps,
            tc.tile_pool(name="rsps", bufs=2, space="PSUM") as rsps,
            tc.tile_pool(name="oh", bufs=3) as ohpool,
            tc.tile_pool(name="combo", bufs=3) as combopool,
        ):
            # ---------- constants / small inputs ----------
            # idtr = [I | transitions]  (fp32) for the final Frobenius
            idtr = consts.tile([C, 2 * C], f32)
            nc.sync.dma_start(out=idtr[:, 0:C], in_=ident_in[:])
            nc.sync.dma_start(out=idtr[:, C : 2 * C], in_=trans_in[:])
            E_bf = consts.tile([C, C], bf16)
            nc.scalar.activation(out=E_bf[:], in_=idtr[:, C : 2 * C], func=Act.Exp)
            transT_sb = consts.tile([C, C], f32)
            nc.sync.dma_start(out=transT_sb[:], in_=transT_in[:])
            ET_bf = consts.tile([C, C], bf16)
            nc.scalar.activation(out=ET_bf[:], in_=transT_sb[:], func=Act.Exp)
            ones_bf = consts.tile([C, 1], bf16)
            nc.sync.dma_start(out=ones_bf[:], in_=onesb_in[:])
            ones_row = consts.tile([1, C], bf16)
            nc.sync.dma_start(out=ones_row[:], in_=onesrow_in[:])
            ones_f = consts.tile([C, 1], f32)
            nc.sync.dma_start(out=ones_f[:], in_=onesf_in[:])
            iota4_sb = consts.tile([C, NCH, C], f32)
            nc.sync.dma_start(out=iota4_sb[:], in_=iota4_in[:])
            neg_shift = consts.tile([C, 1], f32)
            nc.vector.memset(neg_shift[:], -SHIFT)

            tags_m = consts.tile([128, NCH, BL], f32)
            nc.sync.dma_start(
                out=tags_m[:],
                in_=tagsm_tb[:].rearrange("(h l) b -> l h b", l=128),
            )
            tags_ms = consts.tile([128, NCH, BL], f32)
            nc.sync.dma_start(
                out=tags_ms[:],
                in_=tagsms_tb[:].rearrange("(h l) b -> l h b", l=128),
            )
            # 1 - mask for t in [TH, T), one partition (K=1 rank-1 matmul rhs)
            onem_sb = consts.tile([1, T - TH, BL], bf16)
            nc.sync.dma_start(out=onem_sb[:], in_=onem_tb[:])

            # ---------- emissions: exp(em - SHIFT), bf16 ----------
            exp_em = bigbuf.tile([C, T, BL], bf16)
            nc.sync.dma_start(out=exp_em[:], in_=em_ctb[:])
            TCH = 64
            for k in range(T // TCH):
                nc.scalar.activation(
                    out=exp_em[:, k * TCH : (k + 1) * TCH, :],
                    in_=exp_em[:, k * TCH : (k + 1) * TCH, :],
                    func=Act.Exp, bias=neg_shift[:], scale=1.0,
                )

            # ---------- the split scan ----------
            # fwd: a(t) = e_t * (E^T a(t-1)),  t = 1..TH-1,  a(0) = e_0
            # bwd (fused, r never materialized in SBUF):
            #   p(t) = E @ w(t) + 1*(1-m_t)   [two accumulating matmuls]
            #   w(t-1) = p(t) * e~_(t-1)      [one DVE mult, PSUM -> SBUF]
            #   with w(511) = e~_511 (since r(512) = 1)
            fa_prev = None
            ub_prev = None
            for k in range(T - TH):
                tf = 1 + k            # 1 .. 255 (skipped when k == T-TH-1)
                tb = T - 1 - k        # 511 .. 256
                # backward
                if tb == T - 1:
                    w_cur = exp_em[:, T - 1, :]
                else:
                    w = wwpool.tile([C, BL], bf16, tag="w")
                    nc.vector.tensor_tensor(
                        out=w[:], in0=ub_prev[:], in1=exp_em[:, tb, :],
                        op=Alu.mult,
                    )
                    w_cur = w[:]
                ub = bps.tile([C, BL], f32, tag="ub")
                nc.tensor.matmul(
                    ub[:], lhsT=ET_bf[:], rhs=w_cur, start=True, stop=False,
                    skip_group_check=True,
                )
                nc.tensor.matmul(
                    ub[:], lhsT=ones_row[:], rhs=onem_sb[:, tb - TH, :],
                    start=False, stop=True, skip_group_check=True,
                )
                ub_prev = ub
                # forward
                if tf < TH:
                    rhs_f = exp_em[:, 0, :] if tf == 1 else fa_prev[:]
                    uf = fps.tile([C, BL], f32, tag="uf")
                    nc.tensor.matmul(
                        uf[:], lhsT=E_bf[:], rhs=rhs_f, start=True, stop=True
                    )
                    fa_new = fapool.tile([C, BL], bf16, tag="fa")
                    nc.vector.tensor_tensor(
                        out=fa_new[:], in0=uf[:], in1=exp_em[:, tf, :],
                        op=Alu.mult,
                    )
                    fa_prev = fa_new

            # ---------- denominator: per-row dot + log ----------
            # p(TH) = r(TH) exactly (rank-1 term included), so
            # Z_b = <p(TH), a(TH-1)> directly.
            d = consts.tile([C, BL], bf16)
            nc.vector.tensor_tensor(
                out=d[:], in0=ub_prev[:], in1=fa_prev[:], op=Alu.mult
            )
            dot_ps = rsps.tile([1, BL], f32, tag="rs")
            nc.tensor.matmul(
                dot_ps[:], lhsT=ones_bf[:, :1], rhs=d[:], start=True, stop=True
            )
            logd = consts.tile([1, BL], f32)
            nc.scalar.activation(out=logd[:], in_=dot_ps[:], func=Act.Ln)
            den_s = consts.tile([1, 1], f32)
            nc.vector.tensor_reduce(
                out=den_s[:], in_=logd[:], axis=Axis.X, op=Alu.add
            )

            # ---------- numerator: one-hot matmuls ----------
            acc_ps = accps.tile([C, 2 * C], f32)
            for b in range(BL):
                oh = ohpool.tile([128, NCH, C], bf16, tag="oh")
                nc.vector.tensor_tensor(
                    out=oh[:], in0=iota4_sb[:],
                    in1=tags_m[:, :, b : b + 1].to_broadcast([128, NCH, C]),
                    op=Alu.is_equal,
                )
                combo = combopool.tile([128, NCH, 2 * C], bf16, tag="combo")
                nc.sync.dma_start(
                    out=combo[:, :, 0:C],
                    in_=em_btc[b].rearrange("(h l) c -> l h c", l=128),
                )
                nc.vector.tensor_tensor(
                    out=combo[:, :, C : 2 * C], in0=iota4_sb[:],
                    in1=tags_ms[:, :, b : b + 1].to_broadcast([128, NCH, C]),
                    op=Alu.is_equal,
                )
                for ch in range(NCH):
                    i = b * NCH + ch
                    nc.tensor.matmul(
                        acc_ps[:], lhsT=oh[:, ch, :], rhs=combo[:, ch, :],
                        start=(i == 0), stop=(i == BL * NCH - 1),
                        skip_group_check=True,
                    )

            # ---------- numerator frobenius ([I | trans] in one shot) ----------
            frob = consts.tile([C, 2 * C], f32)
            nc.vector.tensor_tensor(
                out=frob[:], in0=acc_ps[:], in1=idtr[:], op=Alu.mult
            )
            num_acc = consts.tile([128, 1], f32)
            nc.vector.tensor_reduce(
                out=num_acc[:], in_=frob[:], axis=Axis.X, op=Alu.add
            )
            num_ps = rsps.tile([1, 1], f32, tag="rs")
            nc.tensor.matmul(
                num_ps[:], lhsT=ones_f[:, :1], rhs=num_acc[:],
                start=True, stop=True,
            )

            # ---------- final scalar ----------
            res_sb = consts.tile([1, 1], f32)
            nc.vector.tensor_tensor(
                out=res_sb[:], in0=den_s[:], in1=num_ps[:], op=Alu.subtract
            )
            nc.sync.dma_start(out=out_d[:], in_=res_sb[:])

    nc.compile()
    return nc


def _prep_inputs(emissions, tags, mask, transitions):
    em = np.asarray(emissions)
    tg = np.asarray(tags).astype(np.int32)
    mk = np.asarray(mask).astype(bool)
    tr = np.ascontiguousarray(np.asarray(transitions), dtype=np.float32)
    trT = np.ascontiguousarray(tr.T)

    in_maps = []
    for core in range(NCORES):
        b0, b1 = core * BL, (core + 1) * BL
        em_c = np.asarray(em[b0:b1], dtype=np.float32)
        mk_c3 = mk[b0:b1][:, :, None]                 # [BL, T, 1]
        em_masked = np.where(mk_c3, em_c, -1000.0).astype(np.float32)
        tg_c = tg[b0:b1].T                            # [T, BL] int32
        mk_c = mk[b0:b1].T.astype(np.float32)         # [T, BL]
        pad_f = np.zeros((1, BL), np.float32)

        # masked tags (+1000 where mask off) for the one-hot builds
        tags_m = (tg_c + 1000.0 * (1.0 - mk_c)).astype(np.float32)
        tg_next = np.vstack([tg_c[1:], np.zeros((1, BL), np.int32)])
        mk_next = np.vstack([mk_c[1:], pad_f])
        tags_ms = (tg_next + 1000.0 * (1.0 - mk_next)).astype(np.float32)

        in_maps.append({
            "em_ctb": np.ascontiguousarray(
                em_masked.transpose(2, 1, 0)
            ).astype(ml_dtypes.bfloat16),
            "em_btc": np.ascontiguousarray(em_c).astype(ml_dtypes.bfloat16),
            "tagsm_tb": np.ascontiguousarray(tags_m),
            "tagsms_tb": np.ascontiguousarray(tags_ms),
            "onem_tb": np.ascontiguousarray(
                1.0 - mk_c[TH:T]
            ).astype(ml_dtypes.bfloat16),
            "trans": tr,
            "transT": trT,
        })
    return in_maps


def kernel(emissions, tags, mask, transitions, _want_results=False, **_run_kw):
    from concourse.bass_utils import run_bass_kernel_spmd

    if "nc" not in _cache:
        _cache["nc"] = _build_program()
    nc = _cache["nc"]

    in_maps = _prep_inputs(emissions, tags, mask, transitions)
    res = run_bass_kernel_spmd(nc, in_maps, core_ids=list(range(NCORES)), **_run_kw)
    total = sum(float(r["out"][0, 0]) for r in res.results)
    lengths_total = int(np.asarray(mask).astype(np.int64).sum())
    out = np.float32((total + SHIFT * lengths_total) / B)
    if _want_results:
        return out, res
    return out


# revision 13
# speedup vs baseline: 2.9399x; 1.1749x over previous
"""CRF loss (forward-algorithm log-partition minus gold-path score) on 8 trn2 cores.

Strategy (data-parallel over B, 32 rows per core):

  Denominator via a split scan that halves the serial-latency chain:
    Z_b = 1^T M_l .. M_1 a_0  (l = len_b - 1, M_t = diag(e_t) E^T,
    e_t = exp(emit_t - SHIFT), E = exp(transitions), a_0 = e_0).
    * forward half  (t = 1..255):   a <- e_t * (E^T a)         [true alpha_255]
    * backward half (t = 511..256): r <- E (e~_t * r) + (1-m_t)
      where e~_t is e_t with masked steps zeroed (host bakes -1000 into
      the masked emissions so exp underflows to 0) and m_t is the mask.
      Masked steps therefore compute r <- 0 + 1, the correct "inactive"
      suffix state, with no select op; the +(1-m_t) enters as a rank-1
      K=1 matmul accumulated into the same PSUM group (issued first so
      it prefetches off the critical path). r never materializes: the
      PSUM->SBUF move doubles as the next step's emission multiply.
    Both chains advance in one fused DVE multiply per step over a
    paired emission layout em_pair[c, k, {fwd,bwd}, b], so each scan
    step is two independent matmuls + ONE tensor_tensor.
    Z_b = <r_256, a_255> per row, one dot; SHIFT contributes exactly
    SHIFT*len_b per row, added back on the host.
  All scan arithmetic is bf16 on the PE/DVE path with fp32 PSUM.

  Numerator: one-hot matmuls, one per (b, t-chunk): lhsT = OH(tags),
  rhs = [emissions_chunk | OH(tags_next)] concatenated [128, 256],
  accumulated over all 128 iterations into a single PSUM tile; a single
  Frobenius product with [I | transitions] then yields emit + trans
  scores summed.

Output per core: scalar sum over its rows of (log Z~_b - log_num_b);
host adds SHIFT*sum(len)/B and divides by B.
"""

import numpy as np
import ml_dtypes

B, T, C = 256, 512, 128
NCORES = 8
BL = B // NCORES
TH = T // 2           # split point: fwd covers t<TH via alpha, bwd t>=TH
K = T - TH            # scan iterations (256)
SHIFT = float(np.log(128.0) + 0.5)  # cancels E[log sum_j exp(em_j)] per step
NCH = T // 128        # 4 numerator t-chunks

_cache = {}


def _build_program():
    import concourse.bass as bass
    import concourse.bacc as bacc
    import concourse.tile as tile
    from concourse import mybir

    f32 = mybir.dt.float32
    bf16 = mybir.dt.bfloat16
    Alu = mybir.AluOpType
    Act = mybir.ActivationFunctionType
    Axis = mybir.AxisListType

    nc = bacc.Bacc(None)

    em_pair = nc.dram_tensor("em_pair", [C, K, 2, BL], bf16, kind="ExternalInput")
    em_btc = nc.dram_tensor("em_btc", [BL, T, C], bf16, kind="ExternalInput")
    tagsm_tb = nc.dram_tensor("tagsm_tb", [T, BL], f32, kind="ExternalInput")
    tagsms_tb = nc.dram_tensor("tagsms_tb", [T, BL], f32, kind="ExternalInput")
    onem_tb = nc.dram_tensor("onem_tb", [K, BL], bf16, kind="ExternalInput")
    trans_in = nc.dram_tensor("trans", [C, C], f32, kind="ExternalInput")
    transT_in = nc.dram_tensor("transT", [C, C], f32, kind="ExternalInput")
    out_d = nc.dram_tensor("out", [1, 1], f32, kind="ExternalOutput")

    ident_in = nc.inline_tensor(np.eye(C, dtype=np.float32), name="ident")
    onesb_in = nc.inline_tensor(
        np.ones((C, 1), ml_dtypes.bfloat16), name="onesbf"
    )
    onesrow_in = nc.inline_tensor(
        np.ones((1, C), ml_dtypes.bfloat16), name="onesrow"
    )
    onesf_in = nc.inline_tensor(np.ones((C, 1), np.float32), name="onesf")
    iota4_in = nc.inline_tensor(
        np.broadcast_to(
            np.arange(C, dtype=np.float32), (C, NCH, C)
        ).copy(),
        name="iota4",
    )

    with tile.TileContext(nc) as tc:
        with (
            tc.tile_pool(name="consts", bufs=1) as consts,
            tc.tile_pool(name="bigbuf", bufs=1) as bigbuf,
            tc.tile_pool(name="sp", bufs=3) as sppool,
            tc.tile_pool(name="ups", bufs=2, space="PSUM") as ups,
            tc.tile_pool(name="accps", bufs=1, space="PSUM") as accps,
            tc.tile_pool(name="rsps", bufs=2, space="PSUM") as rsps,
            tc.tile_pool(name="oh", bufs=3) as ohpool,
            tc.tile_pool(name="combo", bufs=3) as combopool,
        ):
            # ---------- constants / small inputs ----------
            # idtr = [I | transitions]  (fp32) for the final Frobenius
            idtr = consts.tile([C, 2 * C], f32)
            nc.sync.dma_start(out=idtr[:, 0:C], in_=ident_in[:])
            nc.sync.dma_start(out=idtr[:, C : 2 * C], in_=trans_in[:])
            E_bf = consts.tile([C, C], bf16)
            nc.scalar.activation(out=E_bf[:], in_=idtr[:, C : 2 * C], func=Act.Exp)
            transT_sb = consts.tile([C, C], f32)
            nc.sync.dma_start(out=transT_sb[:], in_=transT_in[:])
            ET_bf = consts.tile([C, C], bf16)
            nc.scalar.activation(out=ET_bf[:], in_=transT_sb[:], func=Act.Exp)
            ones_bf = consts.tile([C, 1], bf16)
            nc.sync.dma_start(out=ones_bf[:], in_=onesb_in[:])
            ones_row = consts.tile([1, C], bf16)
            nc.sync.dma_start(out=ones_row[:], in_=onesrow_in[:])
            ones_f = consts.tile([C, 1], f32)
            nc.sync.dma_start(out=ones_f[:], in_=onesf_in[:])
            iota4_sb = consts.tile([C, NCH, C], f32)
            nc.sync.dma_start(out=iota4_sb[:], in_=iota4_in[:])
            neg_shift = consts.tile([C, 1], f32)
            nc.vector.memset(neg_shift[:], -SHIFT)

            tags_m = consts.tile([128, NCH, BL], f32)
            nc.sync.dma_start(
                out=tags_m[:],
                in_=tagsm_tb[:].rearrange("(h l) b -> l h b", l=128),
            )
            tags_ms = consts.tile([128, NCH, BL], f32)
            nc.sync.dma_start(
                out=tags_ms[:],
                in_=tagsms_tb[:].rearrange("(h l) b -> l h b", l=128),
            )
            # 1 - mask for t in [TH, T): index j holds t = T-1-j (K=1 rhs)
            onem_sb = consts.tile([1, K, BL], bf16)
            nc.sync.dma_start(out=onem_sb[:], in_=onem_tb[:])

            # ---------- emissions: exp(em - SHIFT), bf16, paired layout ----
            # exp_em[:, k, 0, :] = e_(k+1)   (fwd multiplier at iter k)
            # exp_em[:, k, 1, :] = e~_(510-k) (bwd multiplier at iter k)
            # exp_em[:, K-1, 0, :] = e_0 ; exp_em[:, K-1, 1, :] = e~_511
            exp_em = bigbuf.tile([C, K, 2, BL], bf16)
            # init slice first (chain heads), then chunks in k order
            chunks = [(K - 1, K), (0, 64), (64, 128), (128, 192), (192, K - 1)]
            for lo, hi in chunks:
                nc.sync.dma_start(
                    out=exp_em[:, lo:hi, :, :], in_=em_pair[:, lo:hi, :, :]
                )
                nc.scalar.activation(
                    out=exp_em[:, lo:hi, :, :],
                    in_=exp_em[:, lo:hi, :, :],
                    func=Act.Exp, bias=neg_shift[:], scale=1.0,
                )

            # ---------- the fused split scan ----------
            sp_prev = None
            up_last = None
            for k in range(K):
                up = ups.tile([C, 2, BL], f32, tag="up")
                # bwd: p = E (e~*r) + (1-m);  rank-1 first (prefetches)
                nc.tensor.matmul(
                    up[:, 1, :], lhsT=ones_row[:], rhs=onem_sb[:, k, :],
                    start=True, stop=False, skip_group_check=True,
                )
                rhs_b = (
                    exp_em[:, K - 1, 1, :] if k == 0 else sp_prev[:, 1, :]
                )
                nc.tensor.matmul(
                    up[:, 1, :], lhsT=ET_bf[:], rhs=rhs_b,
                    start=False, stop=True, skip_group_check=True,
                )
                if k < K - 1:
                    # fwd: u = E^T a
                    rhs_f = (
                        exp_em[:, K - 1, 0, :] if k == 0 else sp_prev[:, 0, :]
                    )
                    nc.tensor.matmul(
                        up[:, 0, :], lhsT=E_bf[:], rhs=rhs_f,
                        start=True, stop=True, skip_group_check=True,
                    )
                    # one fused multiply advances both chains
                    sp = sppool.tile([C, 2, BL], bf16, tag="sp")
                    nc.vector.tensor_tensor(
                        out=sp[:], in0=up[:], in1=exp_em[:, k, :, :],
                        op=Alu.mult,
                    )
                    sp_prev = sp
                else:
                    up_last = up

            # ---------- denominator: per-row dot + log ----------
            # Z_b = <p(TH), a(TH-1)>  (p = r exactly, rank-1 term included)
            d = consts.tile([C, BL], bf16)
            nc.vector.tensor_tensor(
                out=d[:], in0=up_last[:, 1, :], in1=sp_prev[:, 0, :],
                op=Alu.mult,
            )
            dot_ps = rsps.tile([1, BL], f32, tag="rs")
            nc.tensor.matmul(
                dot_ps[:], lhsT=ones_bf[:, :1], rhs=d[:], start=True, stop=True
            )
            logd = consts.tile([1, BL], f32)
            nc.scalar.activation(out=logd[:], in_=dot_ps[:], func=Act.Ln)
            den_s = consts.tile([1, 1], f32)
            nc.vector.tensor_reduce(
                out=den_s[:], in_=logd[:], axis=Axis.X, op=Alu.add
            )

            # ---------- numerator: one-hot matmuls ----------
            acc_ps = accps.tile([C, 2 * C], f32)
            for b in range(BL):
                oh = ohpool.tile([128, NCH, C], bf16, tag="oh")
                nc.vector.tensor_tensor(
                    out=oh[:], in0=iota4_sb[:],
                    in1=tags_m[:, :, b : b + 1].to_broadcast([128, NCH, C]),
                    op=Alu.is_equal,
                )
                combo = combopool.tile([128, NCH, 2 * C], bf16, tag="combo")
                nc.sync.dma_start(
                    out=combo[:, :, 0:C],
                    in_=em_btc[b].rearrange("(h l) c -> l h c", l=128),
                )
                nc.vector.tensor_tensor(
                    out=combo[:, :, C : 2 * C], in0=iota4_sb[:],
                    in1=tags_ms[:, :, b : b + 1].to_broadcast([128, NCH, C]),
                    op=Alu.is_equal,
                )
                for ch in range(NCH):
                    i = b * NCH + ch
                    nc.tensor.matmul(
                        acc_ps[:], lhsT=oh[:, ch, :], rhs=combo[:, ch, :],
                        start=(i == 0), stop=(i == BL * NCH - 1),
                        skip_group_check=True,
                    )

            # ---------- numerator frobenius ([I | trans] in one shot) ----------
            frob = consts.tile([C, 2 * C], f32)
            nc.vector.tensor_tensor(
                out=frob[:], in0=acc_ps[:], in1=idtr[:], op=Alu.mult
            )
            num_acc = consts.tile([128, 1], f32)
            nc.vector.tensor_reduce(
                out=num_acc[:], in_=frob[:], axis=Axis.X, op=Alu.add
            )
            num_ps = rsps.tile([1, 1], f32, tag="rs")
            nc.tensor.matmul(
                num_ps[:], lhsT=ones_f[:, :1], rhs=num_acc[:],
                start=True, stop=True,
            )

            # ---------- final scalar ----------
            res_sb = consts.tile([1, 1], f32)
            nc.vector.tensor_tensor(
                out=res_sb[:], in0=den_s[:], in1=num_ps[:], op=Alu.subtract
            )
            nc.sync.dma_start(out=out_d[:], in_=res_sb[:])

    nc.compile()
    return nc


def _prep_inputs(emissions, tags, mask, transitions):
    em = np.asarray(emissions)
    tg = np.asarray(tags).astype(np.int32)
    mk = np.asarray(mask).astype(bool)
    tr = np.ascontiguousarray(np.asarray(transitions), dtype=np.float32)
    trT = np.ascontiguousarray(tr.T)

    # paired time index: slot0 -> t = k+1 (k<K-1), t=0 at k=K-1
    #                    slot1 -> t = 510-k (k<K-1), t=511 at k=K-1
    t_fwd = np.concatenate([np.arange(1, TH), [0]])
    t_bwd = np.concatenate([np.arange(T - 2, TH - 1, -1), [T - 1]])

    in_maps = []
    for core in range(NCORES):
        b0, b1 = core * BL, (core + 1) * BL
        em_c = np.asarray(em[b0:b1], dtype=np.float32)
        mk_c3 = mk[b0:b1][:, :, None]                 # [BL, T, 1]
        em_masked = np.where(mk_c3, em_c, -1000.0).astype(np.float32)
        em_ctb = em_masked.transpose(2, 1, 0)         # [C, T, BL]
        em_pair = np.stack(
            [em_ctb[:, t_fwd, :], em_ctb[:, t_bwd, :]], axis=2
        )                                             # [C, K, 2, BL]
        tg_c = tg[b0:b1].T                            # [T, BL] int32
        mk_c = mk[b0:b1].T.astype(np.float32)         # [T, BL]
        pad_f = np.zeros((1, BL), np.float32)

        # masked tags (+1000 where mask off) for the one-hot builds
        tags_m = (tg_c + 1000.0 * (1.0 - mk_c)).astype(np.float32)
        tg_next = np.vstack([tg_c[1:], np.zeros((1, BL), np.int32)])
        mk_next = np.vstack([mk_c[1:], pad_f])
        tags_ms = (tg_next + 1000.0 * (1.0 - mk_next)).astype(np.float32)

        # onem_sb[0, j, b] = 1 - mask[t = T-1-j]  (rank-1 rhs at iter k=j)
        onem = (1.0 - mk_c[T - 1 : TH - 1 : -1]).astype(ml_dtypes.bfloat16)

        in_maps.append({
            "em_pair": np.ascontiguousarray(em_pair).astype(ml_dtypes.bfloat16),
            "em_btc": np.ascontiguousarray(em_c).astype(ml_dtypes.bfloat16),
            "tagsm_tb": np.ascontiguousarray(tags_m),
            "tagsms_tb": np.ascontiguousarray(tags_ms),
            "onem_tb": np.ascontiguousarray(onem),
            "trans": tr,
            "transT": trT,
        })
    return in_maps


def kernel(emissions, tags, mask, transitions, _want_results=False, **_run_kw):
    from concourse.bass_utils import run_bass_kernel_spmd

    if "nc" not in _cache:
        _cache["nc"] = _build_program()
    nc = _cache["nc"]

    in_maps = _prep_inputs(emissions, tags, mask, transitions)
    res = run_bass_kernel_spmd(nc, in_maps, core_ids=list(range(NCORES)), **_run_kw)
    total = sum(float(r["out"][0, 0]) for r in res.results)
    lengths_total = int(np.asarray(mask).astype(np.int64).sum())
    out = np.float32((total + SHIFT * lengths_total) / B)
    if _want_results:
        return out, res
    return out


# revision 15
# speedup vs baseline: 3.1955x; 1.0870x over previous
"""CRF loss (forward-algorithm log-partition minus gold-path score) on 8 trn2 cores.

Strategy (data-parallel over B, 32 rows per core):

  Denominator via a split scan that halves the serial-latency chain:
    Z_b = 1^T M_l .. M_1 a_0  (l = len_b - 1, M_t = diag(e_t) E^T,
    e_t = exp(emit_t - SHIFT), E = exp(transitions), a_0 = e_0).
    * forward half  (t = 1..255):   a <- e_t * (E^T a)         [true alpha_255]
    * backward half (t = 511..256): r <- E (e~_t * r) + (1-m_t)
      where e~_t is e_t with masked steps zeroed (host bakes -1000 into
      the masked emissions so exp underflows to 0) and m_t is the mask.
      Masked steps therefore compute r <- 0 + 1, the correct "inactive"
      suffix state, with no select op; the +(1-m_t) enters as a rank-1
      K=1 matmul accumulated into the same PSUM group (issued first so
      it prefetches off the critical path). r never materializes: the
      PSUM->SBUF move doubles as the next step's emission multiply.
    Both chains advance in one fused DVE multiply per step over a
    paired emission layout em_pair[c, k, {fwd,bwd}, b], so each scan
    step is two independent matmuls + ONE tensor_tensor.
    Z_b = <r_256, a_255> per row, one dot; SHIFT contributes exactly
    SHIFT*len_b per row, added back on the host.
  All scan arithmetic is bf16 on the PE/DVE path with fp32 PSUM.

  Numerator: one-hot matmuls, one per (b, t-chunk): lhsT = OH(tags),
  rhs = [emissions_chunk | OH(tags_next)] concatenated [128, 256],
  accumulated over all 128 iterations into a single PSUM tile; a single
  Frobenius product with [I | transitions] then yields emit + trans
  scores summed.

Output per core: scalar sum over its rows of (log Z~_b - log_num_b);
host adds SHIFT*sum(len)/B and divides by B.
"""

import numpy as np
import ml_dtypes

B, T, C = 256, 512, 128
NCORES = 8
BL = B // NCORES
TH = T // 2           # split point: fwd covers t<TH via alpha, bwd t>=TH
K = T - TH            # scan iterations (256)
SHIFT = float(np.log(128.0) + 0.5)  # cancels E[log sum_j exp(em_j)] per step
NCH = T // 128        # 4 numerator t-chunks

_cache = {}


def _build_program():
    import concourse.bass as bass
    import concourse.bacc as bacc
    import concourse.tile as tile
    from concourse import mybir

    f32 = mybir.dt.float32
    bf16 = mybir.dt.bfloat16
    Alu = mybir.AluOpType
    Act = mybir.ActivationFunctionType
    Axis = mybir.AxisListType

    nc = bacc.Bacc(None)

    em_pair = nc.dram_tensor("em_pair", [C, K, 2, BL], bf16, kind="ExternalInput")
    em_btc = nc.dram_tensor("em_btc", [BL, T, C], bf16, kind="ExternalInput")
    tagsm_tb = nc.dram_tensor("tagsm_tb", [T, BL], f32, kind="ExternalInput")
    tagsms_tb = nc.dram_tensor("tagsms_tb", [T, BL], f32, kind="ExternalInput")
    onem_tb = nc.dram_tensor("onem_tb", [K, BL], bf16, kind="ExternalInput")
    trans_in = nc.dram_tensor("trans", [C, C], f32, kind="ExternalInput")
    transT_in = nc.dram_tensor("transT", [C, C], f32, kind="ExternalInput")
    out_d = nc.dram_tensor("out", [1, 1], f32, kind="ExternalOutput")

    ident_in = nc.inline_tensor(np.eye(C, dtype=np.float32), name="ident")
    onesb_in = nc.inline_tensor(
        np.ones((C, 1), ml_dtypes.bfloat16), name="onesbf"
    )
    onesrow_in = nc.inline_tensor(
        np.ones((1, C), ml_dtypes.bfloat16), name="onesrow"
    )
    onesf_in = nc.inline_tensor(np.ones((C, 1), np.float32), name="onesf")
    iota4_in = nc.inline_tensor(
        np.broadcast_to(
            np.arange(C, dtype=np.float32), (C, NCH, C)
        ).copy(),
        name="iota4",
    )

    with tile.TileContext(nc) as tc:
        with (
            tc.tile_pool(name="consts", bufs=1) as consts,
            tc.tile_pool(name="bigbuf", bufs=1) as bigbuf,
            tc.tile_pool(name="sp", bufs=3) as sppool,
            tc.tile_pool(name="ups", bufs=2, space="PSUM") as ups,
            tc.tile_pool(name="accps", bufs=1, space="PSUM") as accps,
            tc.tile_pool(name="rsps", bufs=2, space="PSUM") as rsps,
            tc.tile_pool(name="oh", bufs=3) as ohpool,
            tc.tile_pool(name="combo", bufs=3) as combopool,
        ):
            # ---------- scan-critical constants first ----------
            neg_shift = consts.tile([C, 1], f32)
            nc.vector.memset(neg_shift[:], -SHIFT)
            # idtr = [I | transitions]  (fp32) for the final Frobenius
            idtr = consts.tile([C, 2 * C], f32)
            nc.sync.dma_start(out=idtr[:, C : 2 * C], in_=trans_in[:])
            E_bf = consts.tile([C, C], bf16)
            nc.scalar.activation(out=E_bf[:], in_=idtr[:, C : 2 * C], func=Act.Exp)
            transT_sb = consts.tile([C, C], f32)
            nc.sync.dma_start(out=transT_sb[:], in_=transT_in[:])
            ET_bf = consts.tile([C, C], bf16)
            nc.scalar.activation(out=ET_bf[:], in_=transT_sb[:], func=Act.Exp)
            ones_row = consts.tile([1, C], bf16)
            nc.sync.dma_start(out=ones_row[:], in_=onesrow_in[:])
            # 1 - mask for t in [TH, T): index j holds t = T-1-j (K=1 rhs)
            onem_sb = consts.tile([1, K, BL], bf16)
            nc.sync.dma_start(out=onem_sb[:], in_=onem_tb[:])

            # ---------- emissions: exp(em - SHIFT), bf16, paired layout ----
            # exp_em[:, k, 0, :] = e_(k+1)   (fwd multiplier at iter k)
            # exp_em[:, k, 1, :] = e~_(510-k) (bwd multiplier at iter k)
            # exp_em[:, K-1, 0, :] = e_0 ; exp_em[:, K-1, 1, :] = e~_511
            exp_em = bigbuf.tile([C, K, 2, BL], bf16)
            # init slice first (chain heads), then geometrically growing
            # chunks in k order so the scan can start almost immediately
            chunks = [
                (K - 1, K), (0, 8), (8, 24), (24, 56),
                (56, 120), (120, 184), (184, K - 1),
            ]

            def emit_chunk(lo, hi):
                nc.sync.dma_start(
                    out=exp_em[:, lo:hi, :, :], in_=em_pair[:, lo:hi, :, :]
                )
                nc.scalar.activation(
                    out=exp_em[:, lo:hi, :, :],
                    in_=exp_em[:, lo:hi, :, :],
                    func=Act.Exp, bias=neg_shift[:], scale=1.0,
                )

            for lo, hi in chunks[:4]:
                emit_chunk(lo, hi)

            # ---------- remaining constants ----------
            nc.sync.dma_start(out=idtr[:, 0:C], in_=ident_in[:])
            ones_bf = consts.tile([C, 1], bf16)
            nc.sync.dma_start(out=ones_bf[:], in_=onesb_in[:])
            ones_f = consts.tile([C, 1], f32)
            nc.sync.dma_start(out=ones_f[:], in_=onesf_in[:])
            iota4_sb = consts.tile([C, NCH, C], f32)
            nc.sync.dma_start(out=iota4_sb[:], in_=iota4_in[:])

            tags_m = consts.tile([128, NCH, BL], f32)
            nc.sync.dma_start(
                out=tags_m[:],
                in_=tagsm_tb[:].rearrange("(h l) b -> l h b", l=128),
            )
            tags_ms = consts.tile([128, NCH, BL], f32)
            nc.sync.dma_start(
                out=tags_ms[:],
                in_=tagsms_tb[:].rearrange("(h l) b -> l h b", l=128),
            )

            for lo, hi in chunks[4:]:
                emit_chunk(lo, hi)

            # ---------- the fused split scan ----------
            sp_prev = None
            up_last = None
            for k in range(K):
                up = ups.tile([C, 2, BL], f32, tag="up")
                # bwd: p = E (e~*r) + (1-m);  rank-1 first (prefetches)
                nc.tensor.matmul(
                    up[:, 1, :], lhsT=ones_row[:], rhs=onem_sb[:, k, :],
                    start=True, stop=False, skip_group_check=True,
                )
                rhs_b = (
                    exp_em[:, K - 1, 1, :] if k == 0 else sp_prev[:, 1, :]
                )
                nc.tensor.matmul(
                    up[:, 1, :], lhsT=ET_bf[:], rhs=rhs_b,
                    start=False, stop=True, skip_group_check=True,
                )
                if k < K - 1:
                    # fwd: u = E^T a
                    rhs_f = (
                        exp_em[:, K - 1, 0, :] if k == 0 else sp_prev[:, 0, :]
                    )
                    nc.tensor.matmul(
                        up[:, 0, :], lhsT=E_bf[:], rhs=rhs_f,
                        start=True, stop=True, skip_group_check=True,
                    )
                    # one fused multiply advances both chains
                    sp = sppool.tile([C, 2, BL], bf16, tag="sp")
                    nc.vector.tensor_tensor(
                        out=sp[:], in0=up[:], in1=exp_em[:, k, :, :],
                        op=Alu.mult,
                    )
                    sp_prev = sp
                else:
                    up_last = up

            # ---------- denominator: per-row dot + log ----------
            # Z_b = <p(TH), a(TH-1)>  (p = r exactly, rank-1 term included)
            d = consts.tile([C, BL], bf16)
            nc.vector.tensor_tensor(
                out=d[:], in0=up_last[:, 1, :], in1=sp_prev[:, 0, :],
                op=Alu.mult,
            )
            dot_ps = rsps.tile([1, BL], f32, tag="rs")
            nc.tensor.matmul(
                dot_ps[:], lhsT=ones_bf[:, :1], rhs=d[:], start=True, stop=True
            )
            logd = consts.tile([1, BL], f32)
            nc.scalar.activation(out=logd[:], in_=dot_ps[:], func=Act.Ln)
            den_s = consts.tile([1, 1], f32)
            nc.vector.tensor_reduce(
                out=den_s[:], in_=logd[:], axis=Axis.X, op=Alu.add
            )

            # ---------- numerator: one-hot matmuls ----------
            acc_ps = accps.tile([C, 2 * C], f32)
            for b in range(BL):
                oh = ohpool.tile([128, NCH, C], bf16, tag="oh")
                nc.vector.tensor_tensor(
                    out=oh[:], in0=iota4_sb[:],
                    in1=tags_m[:, :, b : b + 1].to_broadcast([128, NCH, C]),
                    op=Alu.is_equal,
                )
                combo = combopool.tile([128, NCH, 2 * C], bf16, tag="combo")
                nc.sync.dma_start(
                    out=combo[:, :, 0:C],
                    in_=em_btc[b].rearrange("(h l) c -> l h c", l=128),
                )
                nc.vector.tensor_tensor(
                    out=combo[:, :, C : 2 * C], in0=iota4_sb[:],
                    in1=tags_ms[:, :, b : b + 1].to_broadcast([128, NCH, C]),
                    op=Alu.is_equal,
                )
                for ch in range(NCH):
                    i = b * NCH + ch
                    nc.tensor.matmul(
                        acc_ps[:], lhsT=oh[:, ch, :], rhs=combo[:, ch, :],
                        start=(i == 0), stop=(i == BL * NCH - 1),
                        skip_group_check=True,
                    )

            # ---------- numerator frobenius ([I | trans] in one shot) ----------
            frob = consts.tile([C, 2 * C], f32)
            nc.vector.tensor_tensor(
                out=frob[:], in0=acc_ps[:], in1=idtr[:], op=Alu.mult
            )
            num_acc = consts.tile([128, 1], f32)
            nc.vector.tensor_reduce(
                out=num_acc[:], in_=frob[:], axis=Axis.X, op=Alu.add
            )
            num_ps = rsps.tile([1, 1], f32, tag="rs")
            nc.tensor.matmul(
                num_ps[:], lhsT=ones_f[:, :1], rhs=num_acc[:],
                start=True, stop=True,
            )

            # ---------- final scalar ----------
            res_sb = consts.tile([1, 1], f32)
            nc.vector.tensor_tensor(
                out=res_sb[:], in0=den_s[:], in1=num_ps[:], op=Alu.subtract
            )
            nc.sync.dma_start(out=out_d[:], in_=res_sb[:])

    nc.compile()
    return nc


def _prep_inputs(emissions, tags, mask, transitions):
    em = np.asarray(emissions)
    tg = np.asarray(tags).astype(np.int32)
    mk = np.asarray(mask).astype(bool)
    tr = np.ascontiguousarray(np.asarray(transitions), dtype=np.float32)
    trT = np.ascontiguousarray(tr.T)

    # paired time index: slot0 -> t = k+1 (k<K-1), t=0 at k=K-1
    #                    slot1 -> t = 510-k (k<K-1), t=511 at k=K-1
    t_fwd = np.concatenate([np.arange(1, TH), [0]])
    t_bwd = np.concatenate([np.arange(T - 2, TH - 1, -1), [T - 1]])

    in_maps = []
    for core in range(NCORES):
        b0, b1 = core * BL, (core + 1) * BL
        em_c = np.asarray(em[b0:b1], dtype=np.float32)
        mk_c3 = mk[b0:b1][:, :, None]                 # [BL, T, 1]
        em_masked = np.where(mk_c3, em_c, -1000.0).astype(np.float32)
        em_ctb = em_masked.transpose(2, 1, 0)         # [C, T, BL]
        em_pair = np.stack(
            [em_ctb[:, t_fwd, :], em_ctb[:, t_bwd, :]], axis=2
        )                                             # [C, K, 2, BL]
        tg_c = tg[b0:b1].T                            # [T, BL] int32
        mk_c = mk[b0:b1].T.astype(np.float32)         # [T, BL]
        pad_f = np.zeros((1, BL), np.float32)

        # masked tags (+1000 where mask off) for the one-hot builds
        tags_m = (tg_c + 1000.0 * (1.0 - mk_c)).astype(np.float32)
        tg_next = np.vstack([tg_c[1:], np.zeros((1, BL), np.int32)])
        mk_next = np.vstack([mk_c[1:], pad_f])
        tags_ms = (tg_next + 1000.0 * (1.0 - mk_next)).astype(np.float32)

        # onem_sb[0, j, b] = 1 - mask[t = T-1-j]  (rank-1 rhs at iter k=j)
        onem = (1.0 - mk_c[T - 1 : TH - 1 : -1]).astype(ml_dtypes.bfloat16)

        in_maps.append({
            "em_pair": np.ascontiguousarray(em_pair).astype(ml_dtypes.bfloat16),
            "em_btc": np.ascontiguousarray(em_c).astype(ml_dtypes.bfloat16),
            "tagsm_tb": np.ascontiguousarray(tags_m),
            "tagsms_tb": np.ascontiguousarray(tags_ms),
            "onem_tb": np.ascontiguousarray(onem),
            "trans": tr,
            "transT": trT,
        })
    return in_maps


def kernel(emissions, tags, mask, transitions, _want_results=False, **_run_kw):
    from concourse.bass_utils import run_bass_kernel_spmd

    if "nc" not in _cache:
        _cache["nc"] = _build_program()
    nc = _cache["nc"]

    in_maps = _prep_inputs(emissions, tags, mask, transitions)
    res = run_bass_kernel_spmd(nc, in_maps, core_ids=list(range(NCORES)), **_run_kw)
    total = sum(float(r["out"][0, 0]) for r in res.results)
    lengths_total = int(np.asarray(mask).astype(np.int64).sum())
    out = np.float32((total + SHIFT * lengths_total) / B)
    if _want_results:
        return out, res
    return out
